# revision 24
# baseline (speedup 1.0000x reference)
"""ArcFace (AngularPenaltySMLoss) distributed Trainium2 kernel.

Strategy (tensor-parallel over classes, per the sharding hint):
  - Shard W's C=100000 rows over 8 cores (12500 each).
  - Host: normalize x, transpose to xn.T [D, B]; per-core W_shard.T
    [D, C_SHARD] contiguous (contraction dim D lands on SBUF partitions, no
    on-chip transpose). Both pre-scaled and cast to fp8e4m3 (the scales are
    folded back out inside the device exp()).
  - Device (SPMD, no collectives): logits tile = xnT.T @ WT chunk into PSUM
    via DoubleRow fp8 matmuls, ScalarE exp(s*logit) -> per-sample partial
    sum-exp over the local classes. DMA [128, 8] partials out.
  - Host: sum partials over cores, compute the (tiny) per-sample target /
    arccos / log path in f64, return the scalar loss.
"""

import sys

if "/opt/trn_rl_repo" not in sys.path:
    sys.path.insert(0, "/opt/trn_rl_repo")

import ml_dtypes
import numpy as np

import concourse.bass as bass
import concourse.mybir as mybir
from concourse import bacc
from concourse.bass_utils import run_bass_kernel_spmd
from concourse.tile import TileContext

B, C, D = 1024, 100000, 512
S_SCALE, MARGIN, EPS = 64.0, 0.5, 1e-7
N_CORES = 8
C_SHARD = C // N_CORES          # 12500
P = 128
KO = D // P                     # 4 k-chunks of 128
B_TILES = B // P                # 8
CHUNK = 2048                    # classes per PSUM tile (4 banks)
MM_N = 512                      # one matmul output <= one PSUM bank
N_WARM = 18                     # PE warm-up matmuls (HAM runway over the fill)

# fp8e4m3 with pre-scaling to dodge subnormals; exp scale folds it back out.
DTYPE = "fp8"                   # "fp8" | "bf16"
REDUCE = "dve"                  # "accum" (ACT accum_out) | "dve" (exp->bf16, DVE reduce)

_CFG = {
    "bf16": dict(
        mdt=mybir.dt.bfloat16, npdt=ml_dtypes.bfloat16,
        wscale=1.0, xscale=1.0, kstep=1, perf=None,
    ),
    "fp8": dict(
        mdt=mybir.dt.float8e4, npdt=ml_dtypes.float8_e4m3,
        wscale=8.0, xscale=4.0, kstep=2,
        perf=mybir.MatmulPerfMode.DoubleRow,
    ),
}

# chunk list: a small first chunk so the first wt DMA (and first matmuls)
# come up fast, a medium last chunk so the post-matmul exp/reduce tail is
# short, full 2048s in between
def _chunks():
    spans = [(0, 1024)]
    c0 = 1024
    while C_SHARD - c0 > CHUNK:
        spans.append((c0, CHUNK))
        c0 += CHUNK
    spans.append((c0, C_SHARD - c0))
    return spans


LAST_RESULT = None
_NC_CACHE = None


def _build_bass():
    cfg = _CFG[DTYPE]
    mdt, kstep, perf = cfg["mdt"], cfg["kstep"], cfg["perf"]
    act_scale = S_SCALE / (cfg["wscale"] * cfg["xscale"])
    spans = _chunks()
    n_chunks = len(spans)

    nc = bacc.Bacc("TRN2")
    xnt = nc.declare_dram_parameter("xnt", [D, B], mdt, isOutput=False)
    wt = nc.declare_dram_parameter("wt", [D, C_SHARD], mdt, isOutput=False)
    out = nc.declare_dram_parameter("out", [P, B_TILES], mybir.dt.float32, isOutput=True)

    with TileContext(nc) as tc:
        with (
            tc.tile_pool(name="xpool", bufs=1) as xpool,
            tc.tile_pool(name="wpool", bufs=3) as wpool,
            tc.tile_pool(name="epool", bufs=4) as epool,
            tc.tile_pool(name="accp", bufs=1) as accp,
            tc.tile_pool(name="psum", bufs=2, space="PSUM") as psum,
        ):
            # xn.T resident in SBUF (host ships the exact [p, ko, b] layout,
            # so per-partition runs are contiguous). Two DMAs on the
            # Activation HWDGE queue -> two HW queue sets in parallel, while
            # the first wt chunk flows on the Sync queue.
            xnt_sb = xpool.tile([P, KO, B], mdt)
            xnt_r = xnt.rearrange("(ko p) b -> p ko b", p=P)
            nc.scalar.dma_start(xnt_sb[:], xnt_r[:])

            # PE warm-up: HAM un-throttles (1.2 -> 2.4 GHz) only after
            # ~3.4us of sustained matmul activity; these bridge the PE from
            # engine-start to the first data-dependent matmul so the real
            # stream runs warm from the beginning.
            wsrc = xpool.tile([P, MM_N], mdt, tag="warm_src")
            nc.vector.memset(wsrc[:], 1)
            for _ in range(N_WARM):
                pw = psum.tile([P, CHUNK], mybir.dt.float32, tag="ps")
                nc.tensor.matmul(
                    pw[:, :MM_N], wsrc[:, :P], wsrc[:], start=True, stop=True
                )

            # per-(b-tile, chunk) partial sums of exp(s * logit)
            acc = accp.tile([P, B_TILES, n_chunks], mybir.dt.float32)
            out_sb = accp.tile([P, B_TILES], mybir.dt.float32)

            wt_r = wt.rearrange("(ko p) c -> p ko c", p=P)

            for ci, (c0, cw) in enumerate(spans):
                wt_tile = wpool.tile([P, KO, CHUNK], mdt, tag="wt")
                nc.sync.dma_start(wt_tile[:, :, :cw], wt_r[:, :, c0 : c0 + cw])
                wtv = wt_tile

                for bt in range(B_TILES):
                    ps = psum.tile([P, CHUNK], mybir.dt.float32, tag="ps")
                    n_sub = (cw + MM_N - 1) // MM_N
                    for k in range(0, KO, kstep):
                        for si in range(n_sub):
                            s0 = si * MM_N
                            sw = min(MM_N, cw - s0)
                            if kstep == 2:
                                lhsT = xnt_sb[:, k : k + 2, bt * P : (bt + 1) * P]
                                rhs = wtv[:, k : k + 2, s0 : s0 + sw]
                            else:
                                lhsT = xnt_sb[:, k, bt * P : (bt + 1) * P]
                                rhs = wtv[:, k, s0 : s0 + sw]
                            nc.tensor.matmul(
                                ps[:, s0 : s0 + sw],
                                lhsT,
                                rhs,
                                start=(k == 0),
                                stop=(k + kstep >= KO),
                                perf_mode=perf,
                            )
                    # last two chunks' reduces go via ACT accum (cheap: +~300ns
                    # each) so the DVE reduce queue doesn't trail the final MMs
                    use_accum = REDUCE == "accum" or ci >= n_chunks - 2
                    if use_accum:
                        # exp elementwise (in place) + free-dim accumulate
                        nc.scalar.activation(
                            ps[:, :cw],
                            ps[:, :cw],
                            mybir.ActivationFunctionType.Exp,
                            scale=act_scale,
                            accum_out=acc[:, bt, ci : ci + 1],
                        )
                    else:
                        ex = epool.tile([P, CHUNK], mybir.dt.bfloat16, tag="ex")
                        nc.scalar.activation(
                            ex[:, :cw],
                            ps[:, :cw],
                            mybir.ActivationFunctionType.Exp,
                            scale=act_scale,
                        )
                        nc.vector.reduce_sum(
                            acc[:, bt, ci : ci + 1],
                            ex[:, :cw],
                            axis=mybir.AxisListType.X,
                        )

            for bt in range(B_TILES):
                nc.vector.reduce_sum(
                    out_sb[:, bt : bt + 1],
                    acc[:, bt, :],
                    axis=mybir.AxisListType.X,
                )
            nc.sync.dma_start(out[:], out_sb[:])

    nc.compile()
    return nc


def _get_nc():
    global _NC_CACHE
    if _NC_CACHE is None:
        _NC_CACHE = _build_bass()
    return _NC_CACHE


def kernel(x: np.ndarray, labels: np.ndarray, W: np.ndarray) -> np.ndarray:
    global LAST_RESULT
    cfg = _CFG[DTYPE]
    x = np.asarray(x, dtype=np.float32)
    W = np.asarray(W, dtype=np.float32)
    labels = np.asarray(labels)

    # ---- host prep (sharding glue) ----
    norms = np.maximum(np.sqrt((x.astype(np.float64) ** 2).sum(axis=1)), 1e-12)
    xn = (x / norms[:, None].astype(np.float32)).astype(np.float32)
    xnt_q = np.ascontiguousarray(xn.T * cfg["xscale"]).astype(cfg["npdt"])

    in_maps = []
    for i in range(N_CORES):
        shard = W[i * C_SHARD : (i + 1) * C_SHARD]
        wt_q = np.ascontiguousarray(shard.T * cfg["wscale"]).astype(cfg["npdt"])
        in_maps.append({"xnt": xnt_q, "wt": wt_q})

    # ---- device: per-core partial sum over classes of exp(s*logit) ----
    nc = _get_nc()
    res = run_bass_kernel_spmd(nc, in_maps, core_ids=list(range(N_CORES)))
    LAST_RESULT = res

    # ---- host combine (the all-reduce + tiny per-sample tail) ----
    sumexp = np.zeros(B, dtype=np.float64)
    for i in range(N_CORES):
        part = res.results[i]["out"].astype(np.float64)  # [P, B_TILES]
        sumexp += part.T.reshape(B)                      # b = bt*128 + p

    target = np.einsum(
        "bd,bd->b", xn.astype(np.float64), W[labels].astype(np.float64)
    )
    tgt = np.clip(target, -1.0 + EPS, 1.0 - EPS)
    numerator = S_SCALE * np.cos(np.arccos(tgt) + MARGIN)
    excl = sumexp - np.exp(S_SCALE * tgt)
    L = numerator - np.log(np.exp(numerator) + excl)
    return np.array(-L.mean(), dtype=np.float32)
